# revision 33
# baseline (speedup 1.0000x reference)
"""APPNP (MLP + 10 sparse propagation iterations) on 8 Trainium2 NeuronCores.

Design (per sharding hint; all FLOPs on device, host does indexing only):
  - Destination nodes sharded across 8 cores (12500 real + 44 pad slots),
    degree-balanced into 98 blocks of 128 PSUM slots (serpentine bin-pack).
  - Edges partitioned by dest core, grouped by (dest block, source quarter
    of the 32768-row int16 gather-index space), padded to 128-edge chunks
    with a cross-core max K table so all 8 cores share ONE static SPMD
    instruction schedule.
  - Per iteration: dma_gather pulls 256B fp32 source rows from a replicated
    z' copy in HBM (<=63 chunks per gather for the SWDGE ring,
    single_packet=False, sub-tiles of 21 chunks recycle message slots
    early); scatter-add runs as one-hot selection-matrix matmuls into
    per-block PSUM accumulators (S built on-device: 80% DVE iota==dest
    compares in 2x mode, 20% ScalarE Abs/Relu pairs); the combine folds the
    GCN normalization into two per-partition scalings
    (z' = dinv*(0.9*dinv*agg + alpha*h)); ONE AllGather redistributes the
    new z' shards (collectives block the Pool queue on this runtime, so a
    single large collective beats pipelined splits; its output must stay a
    plain Internal tensor - Shared-space tensors crash dma_gather).
  - MLP (h = relu(x@W0+b0)@W1+b1) runs once on-device from a
    host-transposed x shard, transpose-free via out-[64,128] matmuls.
"""

import os
import numpy as np

import concourse.bass as bass
import concourse.bacc as bacc
import concourse.tile as tile
import concourse.mybir as mybir
from concourse.bass_utils import run_bass_kernel_spmd

F32 = mybir.dt.float32
I16 = mybir.dt.int16

N = 100000
F_IN = 512
H = 64
NCORES = 8
ALPHA = 0.1
NITER = int(os.environ.get("APPNP_NITER", "10"))
SKIP = os.environ.get("APPNP_SKIP", "")
ACT_FRAC10 = int(os.environ.get("APPNP_ACT10", "2"))

DPC = N // NCORES          # 12500 real dests per core
NBLK = 98                  # blocks of 128 dest slots
SLOTS = NBLK * 128         # 12544 padded slots per core
SGB = 7                    # blocks per supergroup
NSG = NBLK // SGB          # 14 supergroups
NTOT = NCORES * SLOTS      # 100352
NQ = 4                     # source quarters of the global id space
QSZ = 32768                # quarter size (int16 gather index limit)
GMAX = 63                  # max chunks per dma_gather (SWDGE ring: 128 entries)
SUBC = 21                  # chunks per message sub-tile (early slot recycling)


def _prep_graph(edge_index, edge_weight):
    """Host-side: shard/sort/pad edges; returns per-core data + shared K table."""
    row = edge_index[0].astype(np.int64)
    col = edge_index[1].astype(np.int64)
    loops = np.arange(N, dtype=np.int64)
    row = np.concatenate([row, loops])
    col = np.concatenate([col, loops])
    w = np.concatenate([edge_weight.astype(np.float32), np.ones(N, np.float32)])

    # degrees exactly as the reference: deg = segment_sum(w, row)
    deg = np.bincount(row, weights=w.astype(np.float64), minlength=N)
    deg = deg.astype(np.float32)
    dinv = np.where(deg > 0, 1.0 / np.sqrt(np.maximum(deg, 1e-30)), 0.0).astype(
        np.float32
    )

    perm = _make_perm(col)
    return _prep_graph2(row, col, w, dinv, perm)


def _make_perm(col):
    # slot = perm[core][local_old]; bin-pack dests by in-degree into the
    # 98 blocks (serpentine over degree-sorted dests) to balance blocks.
    indeg = np.bincount(col, minlength=N) + 1
    perm = np.empty((NCORES, DPC), dtype=np.int64)
    if os.environ.get("APPNP_NOBAL"):
        perm[:] = np.arange(DPC)
    else:
        _bal(perm, indeg)
    return perm


def _bal(perm, indeg):
    for c in range(NCORES):
        deg_c = indeg[c * DPC : (c + 1) * DPC]
        order = np.argsort(-deg_c, kind="stable")
        # serpentine round-robin over blocks equalizes block sums
        r = np.arange(DPC)
        rnd = r // NBLK
        bi = r % NBLK
        bi = np.where(rnd % 2 == 1, NBLK - 1 - bi, bi)  # serpentine
        # slot index within block = round number
        pos = bi * 128 + rnd
        perm[c, order] = pos


def _prep_graph2(row, col, w, dinv, perm):
    csrc = row // DPC
    slot_s = perm[csrc, row - csrc * DPC]
    gid = csrc * SLOTS + slot_s
    q = gid // QSZ
    qidx = (gid - q * QSZ).astype(np.int64)
    assert qidx.max() < 32768

    cdst = col // DPC
    ldst = perm[cdst, col - cdst * DPC]
    blk = ldst // 128
    prt = ldst % 128

    # per-core per-(block, quadrant) counts -> shared K table
    key = (cdst * NBLK + blk) * NQ + q
    cnt = np.bincount(key, minlength=NCORES * NBLK * NQ).reshape(NCORES, NBLK, NQ)
    K = np.maximum(1, (cnt.max(axis=0) + 127) // 128).astype(np.int64)  # [NBLK, NQ]

    # chunk/slot layout (shared across cores):
    # emission order: for sg: for qq: for b in sg-blocks: K[b,qq] chunks
    # slot offset table per (b, q)
    slot_off = np.zeros((NBLK, NQ), dtype=np.int64)
    chunk_off = np.zeros((NBLK, NQ), dtype=np.int64)
    instr_C = np.zeros((NSG, NQ), dtype=np.int64)  # chunks per gather instruction
    off = 0
    for sg in range(NSG):
        for qq in range(NQ):
            for b in range(sg * SGB, (sg + 1) * SGB):
                chunk_off[b, qq] = off
                slot_off[b, qq] = off * 128
                off += K[b, qq]
            instr_C[sg, qq] = K[sg * SGB : (sg + 1) * SGB, qq].sum()
    totch = off
    nslots = totch * 128

    # per-core slot arrays
    per_core = []
    for c in range(NCORES):
        m = cdst == c
        eb, eq, ep, eqi, ew = blk[m], q[m], prt[m], qidx[m], w[m]
        order = np.lexsort((ep, eq, eb))
        eb, eq, ep, eqi, ew = (
            eb[order],
            eq[order],
            ep[order],
            eqi[order],
            ew[order],
        )
        # position within (b,q) group
        gkey = eb * NQ + eq
        # edges sorted by gkey; rank within group:
        gstart = np.searchsorted(gkey, np.arange(NBLK * NQ))
        rank = np.arange(len(eb)) - gstart[gkey]
        slots = slot_off[eb, eq] + rank

        sidx = np.zeros(nslots, dtype=np.int16)  # gather index (pad -> 0)
        sdst = np.full(nslots, 999.0, dtype=np.float32)  # S compare val (pad -> 999)
        sw = np.zeros(nslots, dtype=np.float32)  # edge weight (pad -> 0)
        sidx[slots] = eqi.astype(np.int16)
        sdst[slots] = ep.astype(np.float32)
        sw[slots] = ew
        per_core.append((sidx, sdst, sw))

    allones = bool(np.all(w == 1.0))
    return dinv, K, chunk_off, instr_C, totch, per_core, perm, allones

_SENTINEL = None


def _pack_gidx(sidx, instr_C):
    """Pack int16 gather indices into [16, totch*8].

    The SWDGE wraps indices in 16 partitions, replicated 8x across the
    Q7 cores: index i of instruction j (chunk offset coff) lands at
    [i%16 + 16*k, coff*8 + i//16] for k in 0..8.
    """
    totch = len(sidx) // 128
    out = np.zeros((16, totch * 8), dtype=np.int16)
    pos = 0
    coff = 0
    for j in range(NSG * NQ):
        c = int(instr_C.reshape(-1)[j])
        n = c * 128
        vals = sidx[pos : pos + n]
        i = np.arange(n)
        out[i % 16, coff * 8 + (i // 16)] = vals
        pos += n
        coff += c
    assert pos == len(sidx)
    return np.tile(out, (8, 1))


def _build_program(K, chunk_off, instr_C, totch, allones=True):
    """Build the SPMD bass program (same for all cores)."""
    nc = bacc.Bacc("TRN2", target_bir_lowering=False, debug=False, num_devices=NCORES)

    # ---- I/O ----
    xT = nc.dram_tensor("xT", [F_IN, SLOTS], F32, kind="ExternalInput")
    W0c = nc.dram_tensor("W0c", [4, 128, H], F32, kind="ExternalInput")
    W1 = nc.dram_tensor("W1", [H, H], F32, kind="ExternalInput")
    b0c = nc.dram_tensor("b0c", [H, 1], F32, kind="ExternalInput")
    b1r = nc.dram_tensor("b1r", [128, H], F32, kind="ExternalInput")
    coef = nc.dram_tensor("coef", [128, 2, NBLK], F32, kind="ExternalInput")
    iota_d = nc.dram_tensor("iota", [128, 128], F32, kind="ExternalInput")
    gidx_d = nc.dram_tensor("gidx", [128, totch * 8], I16, kind="ExternalInput")
    destv_d = nc.dram_tensor("destv", [128, totch], F32, kind="ExternalInput")
    ndestv_d = nc.dram_tensor("ndestv", [128, totch], F32, kind="ExternalInput")
    if not allones:
        wv_d = nc.dram_tensor("wv", [128, totch], F32, kind="ExternalInput")
    zout = nc.dram_tensor("zout", [SLOTS, H], F32, kind="ExternalOutput")

    # internal DRAM: replicated z' buffers (double buffered)
    zp = [
        nc.dram_tensor(f"zp{i}", [NTOT, H], F32)
        for i in range(2)
    ]
    agb = nc.dram_tensor("agb", [SLOTS, H], F32)

    cmax = int(instr_C.max())

    with tile.TileContext(nc) as tc:
        with (
            tc.tile_pool(name="res", bufs=1) as res,
            tc.tile_pool(name="msg", bufs=22 if allones else 18) as msgp,
            tc.tile_pool(name="sp", bufs=12) as sp,
            tc.tile_pool(name="outp", bufs=3) as outp,
            tc.tile_pool(name="psum", bufs=4, space="PSUM") as psp,
        ):
            # ---- residents ----
            iota_sb = res.tile([128, 128], F32)
            nc.sync.dma_start(out=iota_sb[:], in_=iota_d[:])
            ndestv_sb = res.tile([128, totch], F32)
            nc.sync.dma_start(out=ndestv_sb[:], in_=ndestv_d[:])
            if not allones:
                wv_sb = res.tile([128, totch], F32)
                nc.sync.dma_start(out=wv_sb[:], in_=wv_d[:])
            destv_sb = res.tile([128, totch], F32)
            nc.sync.dma_start(out=destv_sb[:], in_=destv_d[:])
            coef_sb = res.tile([128, 2, NBLK], F32)
            nc.sync.dma_start(out=coef_sb[:], in_=coef[:])
            c1_sb = coef_sb[:, 0, :]
            dinv_sb = coef_sb[:, 1, :]
            ah_sb = res.tile([128, NBLK, H], F32)  # alpha*h
            w0_sb = res.tile([128, 4, H], F32)
            nc.sync.dma_start(out=w0_sb[:], in_=W0c.ap().rearrange("k p h -> p k h"))
            w1_sb = res.tile([H, H], F32)
            nc.sync.dma_start(out=w1_sb[:], in_=W1[:])
            b0_sb = res.tile([H, 1], F32)
            nc.sync.dma_start(out=b0_sb[:], in_=b0c[:])
            b1_sb = res.tile([128, H], F32)
            nc.sync.dma_start(out=b1_sb[:], in_=b1r[:])

            def fire_ag(buf):
                # One big AllGather per iteration: the collective blocks the
                # Pool queue for its whole duration (walrus only allows
                # DMA/Pool engines), so fewer, larger collectives minimize
                # the serialized time.
                nc.gpsimd.collective_compute(
                    "AllGather",
                    mybir.AluOpType.bypass,
                    replica_groups=[list(range(NCORES))],
                    ins=[agb.ap().opt()],
                    outs=[zp[buf].ap().opt()],
                )

            # ---- MLP: h = relu(x@W0+b0)@W1 + b1; write z'_0 = dinv*h ----
            xT_r = xT.ap().rearrange("(k p) c -> p k c", p=128)  # [128, 4, SLOTS]
            with (
                tc.tile_pool(name="mlp", bufs=3) as mlp,
                tc.tile_pool(name="mpsum", bufs=2, space="PSUM") as mpsum,
            ):
                for sg in range(NSG):
                    zslab = outp.tile([128, SGB, H], F32, tag="zslab")
                    for j in range(SGB):
                        b = sg * SGB + j
                        xt = mlp.tile([128, 4, 128], F32, tag="xt")
                        nc.sync.dma_start(
                            out=xt[:], in_=xT_r[:, :, b * 128 : (b + 1) * 128]
                        )
                        ph1 = mpsum.tile([H, 128], F32, tag="ph1")
                        for k in range(4):
                            nc.tensor.matmul(
                                ph1[:],
                                w0_sb[:, k, :],
                                xt[:, k, :],
                                start=(k == 0),
                                stop=(k == 3),
                            )
                        h1T = mlp.tile([H, 128], F32, tag="h1T")
                        nc.scalar.activation(
                            h1T[:],
                            ph1[:],
                            mybir.ActivationFunctionType.Relu,
                            bias=b0_sb[:, 0:1],
                        )
                        ph2 = mpsum.tile([128, H], F32, tag="ph2")
                        nc.tensor.matmul(ph2[:], h1T[:], w1_sb[:], start=True, stop=True)
                        ht = mlp.tile([128, H], F32, tag="ht")
                        nc.vector.tensor_tensor(
                            ht[:], ph2[:], b1_sb[:], mybir.AluOpType.add
                        )
                        nc.vector.tensor_scalar_mul(
                            ah_sb[:, b, :], ht[:], ALPHA
                        )
                        nc.vector.tensor_scalar_mul(
                            zslab[:, j, :], ht[:], dinv_sb[:, b : b + 1]
                        )
                    mlp_dst = zout.ap() if NITER == 0 else agb.ap()
                    nc.sync.dma_start(
                        out=mlp_dst.rearrange("(b p) h -> p b h", p=128)[
                            :, sg * SGB : (sg + 1) * SGB, :
                        ],
                        in_=zslab[:],
                    )
                    if NITER > 0 and sg == NSG - 1:
                        fire_ag(0)

            # ---- propagation iterations ----
            for it in range(NITER):
                last = it == NITER - 1
                for sg in range(NSG):
                    acc = psp.tile([128, SGB * H], F32, name="acc", tag="acc")
                    msgs = {}
                    for qq in range(NQ):
                        C = int(instr_C[sg, qq])
                        coff = int(chunk_off[sg * SGB, qq])
                        gx = sp.tile([128, cmax * 8], I16, name="gx", tag="gx", bufs=3)
                        nc.sync.dma_start(
                            out=gx[:, : C * 8],
                            in_=gidx_d[:, coff * 8 : (coff + C) * 8],
                        )
                        src_ap = zp[it % 2].ap()[
                            qq * QSZ : min((qq + 1) * QSZ, NTOT), :
                        ]
                        subs = []
                        for c0 in range(0, C, SUBC):
                            c1 = min(c0 + SUBC, C)
                            mt = msgp.tile(
                                [128, SUBC, H], F32, name="mt", tag="msg"
                            )
                            if SKIP != "gather":
                                nc.gpsimd.dma_gather(
                                    mt[:, : c1 - c0, :],
                                    src_ap,
                                    gx[:, c0 * 8 : c1 * 8],
                                    (c1 - c0) * 128,
                                    (c1 - c0) * 128,
                                    H,
                                    single_packet=False,
                                )
                            else:
                                nc.vector.memset(mt[:, 0:1, :], 0.0)
                            subs.append((c0, c1, mt))
                        msgs[qq] = subs
                    # matmuls: block-major so PSUM accumulation groups
                    # open/close sequentially within each acc tile
                    for j2 in range(SGB):
                        if SKIP == "mm":
                            break
                        b = sg * SGB + j2
                        a = acc[:, j2 * H : (j2 + 1) * H]
                        for qq in range(NQ):
                            subs = msgs[qq]
                            loc = int(chunk_off[b, qq]) - int(
                                chunk_off[sg * SGB, qq]
                            )
                            for ck in range(int(K[b, qq])):
                                t = int(chunk_off[b, qq]) + ck
                                lc = loc + ck
                                mt = subs[lc // SUBC][2]
                                lc = lc - subs[lc // SUBC][0]
                                if not allones:
                                    nc.vector.tensor_scalar_mul(
                                        mt[:, lc, :],
                                        mt[:, lc, :],
                                        wv_sb[:, t : t + 1],
                                    )
                                st = sp.tile([128, 128], F32, tag="S")
                                if t % 10 < ACT_FRAC10:
                                    nc.scalar.activation(
                                        st[:],
                                        iota_sb[:],
                                        mybir.ActivationFunctionType.Abs,
                                        bias=ndestv_sb[:, t : t + 1],
                                    )
                                    nc.scalar.activation(
                                        st[:],
                                        st[:],
                                        mybir.ActivationFunctionType.Relu,
                                        bias=1.0,
                                        scale=-1.0,
                                    )
                                else:
                                    nc.vector.tensor_scalar(
                                        st[:],
                                        iota_sb[:],
                                        destv_sb[:, t : t + 1],
                                        None,
                                        mybir.AluOpType.is_equal,
                                    )
                                nc.tensor.matmul(
                                    a,
                                    st[:],
                                    mt[:, lc, :],
                                    start=(qq == 0 and ck == 0),
                                    stop=(qq == NQ - 1 and ck == int(K[b, qq]) - 1),
                                )
                    # combine + write: z = 0.9*dinv*agg + alpha*h;
                    # next z' = dinv * z
                    zslab = outp.tile([128, SGB, H], F32, tag="zslab")
                    for j2 in range(SGB):
                        b = sg * SGB + j2
                        a = acc[:, j2 * H : (j2 + 1) * H]
                        tmp = outp.tile([128, H], F32, tag="ctmp")
                        if SKIP == "mm":
                            nc.vector.memset(tmp[:], 0.0)
                        else:
                            nc.vector.tensor_scalar_mul(tmp[:], a, c1_sb[:, b : b + 1])
                        if last:
                            nc.vector.tensor_tensor(
                                zslab[:, j2, :], tmp[:], ah_sb[:, b, :],
                                mybir.AluOpType.add,
                            )
                        else:
                            nc.vector.tensor_tensor(
                                tmp[:], tmp[:], ah_sb[:, b, :],
                                mybir.AluOpType.add,
                            )
                            nc.vector.tensor_scalar_mul(
                                zslab[:, j2, :], tmp[:], dinv_sb[:, b : b + 1]
                            )
                    dst_ap = (zout if last else agb).ap().rearrange(
                        "(b p) h -> p b h", p=128
                    )[:, sg * SGB : (sg + 1) * SGB, :]
                    nc.sync.dma_start(out=dst_ap, in_=zslab[:])
                    if not last and sg == NSG - 1:
                        fire_ag((it + 1) % 2)

    nc.compile()
    return nc


def kernel(x, edge_index, edge_weight, W0, b0, W1, b1):
    x = np.asarray(x, dtype=np.float32)
    dinv, K, chunk_off, instr_C, totch, per_core, perm, allones = _prep_graph(
        np.asarray(edge_index), np.asarray(edge_weight)
    )

    # pack per-core tensors
    in_maps = []
    for c in range(NCORES):
        sidx, sdst, sw = per_core[c]
        g = _pack_gidx(sidx, instr_C)

        destv = sdst.reshape(totch, 128).T.copy()  # [128, totch]

        xs = np.zeros((SLOTS, F_IN), dtype=np.float32)
        xs[perm[c]] = x[c * DPC : (c + 1) * DPC]
        xT = np.ascontiguousarray(xs.T)  # [F_IN, SLOTS]

        dv = np.zeros(SLOTS, dtype=np.float32)
        dv[perm[c]] = dinv[c * DPC : (c + 1) * DPC]
        dv2 = dv.reshape(NBLK, 128).T  # [128, NBLK]
        coef = np.ascontiguousarray(
            np.stack([(1.0 - ALPHA) * dv2, dv2]).transpose(1, 0, 2)
        ).astype(np.float32)

        in_maps.append(
            {
                "xT": xT,
                "W0c": np.asarray(W0, np.float32).reshape(4, 128, H).copy(),
                "W1": np.asarray(W1, np.float32),
                "b0c": np.asarray(b0, np.float32).reshape(H, 1).copy(),
                "b1r": np.broadcast_to(
                    np.asarray(b1, np.float32), (128, H)
                ).copy(),
                "coef": coef,
                "iota": np.broadcast_to(
                    np.arange(128, dtype=np.float32), (128, 128)
                ).copy(),
                "gidx": g,
                "destv": destv,
                "ndestv": -destv,
                **(
                    {}
                    if allones
                    else {"wv": sw.reshape(totch, 128).T.copy()}
                ),
            }
        )

    nc = _build_program(K, chunk_off, instr_C, totch, allones)
    res = run_bass_kernel_spmd(nc, in_maps, core_ids=list(range(NCORES)))
    nbench = int(os.environ.get("APPNP_BENCH", "0"))
    if nbench:
        import time as _time
        for i in range(nbench):
            t0 = _time.perf_counter()
            run_bass_kernel_spmd(nc, in_maps, core_ids=list(range(NCORES)))
            print(f"bench run {i}: {_time.perf_counter()-t0:.3f}s", flush=True)

    global LAST_PERM, LAST_NC
    LAST_PERM = perm
    LAST_NC = nc
    out = np.empty((N, H), dtype=np.float32)
    for c in range(NCORES):
        out[c * DPC : (c + 1) * DPC] = res.results[c]["zout"][perm[c]]
    return out
